# revision 5
# baseline (speedup 1.0000x reference)
"""Binarized-weight MLP (BiMlp, 1w32a adaptive scaling) on 8 TRN2 NeuronCores.

Reference math (per token row x_t of [12544, 1024]):
    bw1 = mean(|w1|,axis=1,keepdims) * sign(w1)        # [4096, 1024]
    h   = gelu(x @ bw1.T + b1)                         # exact (erf) gelu
    bw2 = mean(|w2|,axis=1,keepdims) * sign(w2)        # [1024, 4096]
    out = h @ bw2.T + b2

Strategy: pure data-parallel over the 12544 tokens (1568/core, no collectives;
weights replicated — they fit in SBUF). All compute in a transposed layout
(hT = [H, tokens]) so per-output-channel alpha/bias ride the partition dim and
fuse into a single ScalarE activation (out = gelu(alpha*psum + bias)).
Matmuls run bf16 (sign weights are exact +-1 in bf16) accumulating fp32 in
PSUM; binarization (sign / |w| / alpha = mean|w|) is done on-device.

Host side only reshapes/transposes/shards (layout, not math) and reassembles.
"""

import os
import sys
import types

import numpy as np

N_CORES = 8
B, S, D, H = 64, 196, 1024, 4096
T_GLOBAL = B * S            # 12544 tokens
T = T_GLOBAL // N_CORES     # 1568 tokens per core
N_CHUNK = 4
TC = T // N_CHUNK           # 392 token columns per matmul (<=512 psum bank)
KD = D // 128               # 8 k-tiles over D
KH = H // 128               # 32 k-tiles over H


def _install_ntff_hook():
    """This container image lacks antenv.axon_hooks; synthesize it so
    run_bass_kernel_spmd(trace=True) can capture NTFF profiles through the
    libaxon_pjrt C ABI (the same hook trn_boot would register)."""
    if "antenv.axon_hooks" in sys.modules:
        return
    import contextlib
    import ctypes

    try:
        lib = ctypes.CDLL("/opt/axon/libaxon_pjrt.so")
        lib.axon_start_nrt_profile.argtypes = [
            ctypes.POINTER(ctypes.c_int64),
            ctypes.c_size_t,
        ]
        lib.axon_start_nrt_profile.restype = ctypes.c_int64
        lib.axon_stop_nrt_profile.argtypes = [ctypes.c_char_p]
        lib.axon_stop_nrt_profile.restype = ctypes.c_int64
    except (OSError, AttributeError):
        return

    @contextlib.contextmanager
    def _hook(output_dir, device_ids):
        import jax

        jax.devices()
        if device_ids:
            ids = (ctypes.c_int64 * len(device_ids))(*device_ids)
            rc = lib.axon_start_nrt_profile(ids, len(device_ids))
        else:
            rc = lib.axon_start_nrt_profile(None, 0)
        if rc != 0:
            raise RuntimeError(f"axon_start_nrt_profile rc={rc}")
        try:
            yield
        finally:
            n = lib.axon_stop_nrt_profile(str(output_dir).encode())
            print(f"ntff profile: {n} file(s) in {output_dir}", file=sys.stderr)

    mod = types.ModuleType("antenv.axon_hooks")
    mod.get_axon_ntff_profile_hook = lambda: _hook
    mod.set_axon_ntff_profile_hook = lambda h: None
    sys.modules["antenv.axon_hooks"] = mod


_install_ntff_hook()

import concourse.mybir as mybir  # noqa: E402
from concourse import bacc, tile  # noqa: E402
from concourse.bass import ts  # noqa: E402
from concourse.bass_utils import run_bass_kernel_spmd  # noqa: E402

F32 = mybir.dt.float32
BF16 = mybir.dt.bfloat16
AF = mybir.ActivationFunctionType
ALU = mybir.AluOpType


def build_kernel():
    nc = bacc.Bacc(
        "TRN2",
        target_bir_lowering=False,
        debug=False,
        enable_asserts=False,
        num_devices=N_CORES,
    )
    xt = nc.dram_tensor("xt", [D, T], F32, kind="ExternalInput").ap()
    w1t = nc.dram_tensor("w1t", [D, H], F32, kind="ExternalInput").ap()
    b1 = nc.dram_tensor("b1", [H], F32, kind="ExternalInput").ap()
    w2t = nc.dram_tensor("w2t", [H, D], F32, kind="ExternalInput").ap()
    b2 = nc.dram_tensor("b2", [D], F32, kind="ExternalInput").ap()
    out = nc.dram_tensor("out", [D, T], F32, kind="ExternalOutput").ap()

    w1t_3d = w1t.rearrange("(k p) h -> k p h", p=128)   # [KD, 128, H]
    w2t_3d = w2t.rearrange("(k p) d -> k p d", p=128)   # [KH, 128, D]
    xt_3d = xt.rearrange("(k p) t -> p k t", p=128)     # [128, KD, T]
    out_3d = out.rearrange("(m p) t -> m p t", p=128)   # [KD, 128, T]

    with tile.TileContext(nc) as tc:
        with (
            tc.tile_pool(name="wb", bufs=1) as wbpool,
            tc.tile_pool(name="consts", bufs=1) as cpool,
        ):
            # persistent binarized weights (bf16 +-1)
            w1b = wbpool.tile([128, KD, H], BF16, tag="w1b")
            w2b = wbpool.tile([128, KH, D], BF16, tag="w2b")
            # per-output-channel constants, partition-major per 128-tile
            alpha1c = cpool.tile([128, KH], F32, tag="a1")
            alpha2c = cpool.tile([128, KD], F32, tag="a2")
            b1c = cpool.tile([128, KH], F32, tag="b1")
            b2c = cpool.tile([128, KD], F32, tag="b2")
            ones1 = cpool.tile([128, 1], F32, tag="ones1")
            ones2 = cpool.tile([128, 1], F32, tag="ones2")

            nc.vector.memset(ones1[:], 1.0 / D)
            nc.vector.memset(ones2[:], 1.0 / H)
            nc.sync.dma_start(b1c[:], b1.rearrange("(m p) -> p m", p=128))
            nc.sync.dma_start(b2c[:], b2.rearrange("(m p) -> p m", p=128))

            # ---- prologue: binarize weights, alpha = mean|w| via PE ----
            with (
                tc.tile_pool(name="stage", bufs=2) as spool,
                tc.tile_pool(name="absw", bufs=2) as apool,
                tc.tile_pool(name="psa", bufs=2, space="PSUM") as psa,
            ):
                with nc.named_scope("w1prep"):
                    a1ps = psa.tile([128, KH], F32, tag="aps")
                    for k in range(KD):
                        st = spool.tile([128, H], F32, tag="wstage")
                        nc.sync.dma_start(st[:], w1t_3d[k])
                        nc.scalar.activation(w1b[:, k, :], st[:], AF.Sign)
                        ab = apool.tile([128, H], F32, tag="absw")
                        nc.vector.tensor_scalar(
                            ab[:].bitcast(mybir.dt.uint32),
                            st[:].bitcast(mybir.dt.uint32),
                            0x7FFFFFFF,
                            None,
                            ALU.bitwise_and,
                        )
                        for m in range(KH):
                            # start=True clears has_written for the WHOLE bank,
                            # so only the very first matmul into this psum tile
                            # may set it; later columns overwrite-on-first-touch
                            # via the per-element has_written bits.
                            nc.tensor.matmul(
                                a1ps[:, m : m + 1],
                                lhsT=ab[:, ts(m, 128)],
                                rhs=ones1[:],
                                start=(k == 0 and m == 0),
                                stop=(k == KD - 1 and m == KH - 1),
                                skip_group_check=True,
                            )
                    nc.vector.tensor_copy(out=alpha1c[:], in_=a1ps[:])

                with nc.named_scope("w2prep"):
                    a2ps = psa.tile([128, KD], F32, tag="aps")
                    for k in range(KH):
                        st2 = spool.tile([128, H], F32, tag="wstage")
                        nc.sync.dma_start(st2[:, :D], w2t_3d[k])
                        nc.scalar.activation(w2b[:, k, :], st2[:, :D], AF.Sign)
                        ab2 = apool.tile([128, H], F32, tag="absw")
                        nc.vector.tensor_scalar(
                            ab2[:, :D].bitcast(mybir.dt.uint32),
                            st2[:, :D].bitcast(mybir.dt.uint32),
                            0x7FFFFFFF,
                            None,
                            ALU.bitwise_and,
                        )
                        for m in range(KD):
                            nc.tensor.matmul(
                                a2ps[:, m : m + 1],
                                lhsT=ab2[:, ts(m, 128)],
                                rhs=ones2[:],
                                start=(k == 0 and m == 0),
                                stop=(k == KH - 1 and m == KD - 1),
                                skip_group_check=True,
                            )
                    nc.vector.tensor_copy(out=alpha2c[:], in_=a2ps[:])

            # ---- main: 4 token chunks, fc1 -> gelu -> fc2 ----
            with (
                tc.tile_pool(name="xc", bufs=2) as xpool,
                tc.tile_pool(name="ht", bufs=1) as hpool,
                tc.tile_pool(name="oc", bufs=3) as opool,
                tc.tile_pool(name="ps1", bufs=3, space="PSUM") as ps1pool,
                tc.tile_pool(name="ps2", bufs=3, space="PSUM") as ps2pool,
            ):
                for c in range(N_CHUNK):
                    csl = slice(c * TC, (c + 1) * TC)
                    with nc.named_scope(f"fc1_c{c}"):
                        xc = xpool.tile([128, KD, TC], BF16, tag="xc")
                        nc.gpsimd.dma_start(xc[:], xt_3d[:, :, csl])
                        ht = hpool.tile([128, KH, TC], BF16, tag="ht")
                        for m in range(KH):
                            ps = ps1pool.tile([128, TC], F32, tag="ps1")
                            for k in range(KD):
                                nc.tensor.matmul(
                                    ps[:],
                                    lhsT=w1b[:, k, ts(m, 128)],
                                    rhs=xc[:, k, :],
                                    start=(k == 0),
                                    stop=(k == KD - 1),
                                )
                            nc.scalar.activation(
                                ht[:, m, :],
                                ps[:],
                                AF.Gelu,
                                bias=b1c[:, m : m + 1],
                                scale=alpha1c[:, m : m + 1],
                            )
                    with nc.named_scope(f"fc2_c{c}"):
                        for md in range(KD):
                            ps2 = ps2pool.tile([128, TC], F32, tag="ps2")
                            for mh in range(KH):
                                nc.tensor.matmul(
                                    ps2[:],
                                    lhsT=w2b[:, mh, ts(md, 128)],
                                    rhs=ht[:, mh, :],
                                    start=(mh == 0),
                                    stop=(mh == KH - 1),
                                )
                            oc = opool.tile([128, TC], F32, tag="oc")
                            nc.scalar.activation(
                                oc[:],
                                ps2[:],
                                AF.Identity,
                                bias=b2c[:, md : md + 1],
                                scale=alpha2c[:, md : md + 1],
                            )
                            nc.sync.dma_start(out_3d[md][:, csl], oc[:])

    nc.compile()
    return nc


_NC_CACHE = None


def _get_nc():
    global _NC_CACHE
    if _NC_CACHE is None:
        _NC_CACHE = build_kernel()
    return _NC_CACHE


def kernel(x, w1, b1, w2, b2):
    assert x.shape == (B, S, D) and w1.shape == (H, D) and w2.shape == (D, H)
    nc = _get_nc()

    xt = np.ascontiguousarray(x.reshape(T_GLOBAL, D).T)      # [D, 12544]
    w1t = np.ascontiguousarray(w1.T)                          # [D, H]
    w2t = np.ascontiguousarray(w2.T)                          # [H, D]
    b1 = np.ascontiguousarray(b1, dtype=np.float32)
    b2 = np.ascontiguousarray(b2, dtype=np.float32)

    in_maps = [
        {
            "xt": np.ascontiguousarray(xt[:, i * T : (i + 1) * T]),
            "w1t": w1t,
            "b1": b1,
            "w2t": w2t,
            "b2": b2,
        }
        for i in range(N_CORES)
    ]

    trace = bool(int(os.environ.get("BIMLP_TRACE", "0")))
    res = run_bass_kernel_spmd(
        nc, in_maps, core_ids=list(range(N_CORES)), trace=trace
    )
    if trace:
        kernel.last_results = res

    outt = np.concatenate([res.results[i]["out"] for i in range(N_CORES)], axis=1)
    return np.ascontiguousarray(outt.T).reshape(B, S, D).astype(np.float32)


# revision 8
# speedup vs baseline: 1.1770x; 1.1770x over previous
"""Binarized-weight MLP (BiMlp, 1w32a adaptive scaling) on 8 TRN2 NeuronCores.

Reference math (per token row x_t of [12544, 1024]):
    bw1 = mean(|w1|,axis=1,keepdims) * sign(w1)        # [4096, 1024]
    h   = gelu(x @ bw1.T + b1)                         # exact (erf) gelu
    bw2 = mean(|w2|,axis=1,keepdims) * sign(w2)        # [1024, 4096]
    out = h @ bw2.T + b2

Strategy: pure data-parallel over the 12544 tokens (1568/core, no collectives;
weights replicated — they fit in SBUF). All compute in a transposed layout
(hT = [H, tokens]) so per-output-channel alpha/bias ride the partition dim and
fuse into a single ScalarE activation (out = gelu(alpha*psum + bias)).
Matmuls run bf16 (sign weights are exact +-1 in bf16) accumulating fp32 in
PSUM; binarization (sign / |w| / alpha = mean|w|) is done on-device.

Host side only reshapes/transposes/shards (layout, not math) and reassembles.
"""

import os
import sys
import types

import numpy as np

N_CORES = 8
B, S, D, H = 64, 196, 1024, 4096
T_GLOBAL = B * S            # 12544 tokens
T = T_GLOBAL // N_CORES     # 1568 tokens per core
N_CHUNK = 4
TC = T // N_CHUNK           # 392 token columns per matmul (<=512 psum bank)
KD = D // 128               # 8 k-tiles over D
KH = H // 128               # 32 k-tiles over H


def _install_ntff_hook():
    """This container image lacks antenv.axon_hooks; synthesize it so
    run_bass_kernel_spmd(trace=True) can capture NTFF profiles through the
    libaxon_pjrt C ABI (the same hook trn_boot would register)."""
    if "antenv.axon_hooks" in sys.modules:
        return
    import contextlib
    import ctypes

    try:
        lib = ctypes.CDLL("/opt/axon/libaxon_pjrt.so")
        lib.axon_start_nrt_profile.argtypes = [
            ctypes.POINTER(ctypes.c_int64),
            ctypes.c_size_t,
        ]
        lib.axon_start_nrt_profile.restype = ctypes.c_int64
        lib.axon_stop_nrt_profile.argtypes = [ctypes.c_char_p]
        lib.axon_stop_nrt_profile.restype = ctypes.c_int64
    except (OSError, AttributeError):
        return

    @contextlib.contextmanager
    def _hook(output_dir, device_ids):
        import jax

        jax.devices()
        if device_ids:
            ids = (ctypes.c_int64 * len(device_ids))(*device_ids)
            rc = lib.axon_start_nrt_profile(ids, len(device_ids))
        else:
            rc = lib.axon_start_nrt_profile(None, 0)
        if rc != 0:
            raise RuntimeError(f"axon_start_nrt_profile rc={rc}")
        try:
            yield
        finally:
            n = lib.axon_stop_nrt_profile(str(output_dir).encode())
            print(f"ntff profile: {n} file(s) in {output_dir}", file=sys.stderr)

    mod = types.ModuleType("antenv.axon_hooks")
    mod.get_axon_ntff_profile_hook = lambda: _hook
    mod.set_axon_ntff_profile_hook = lambda h: None
    sys.modules["antenv.axon_hooks"] = mod


_install_ntff_hook()

import concourse.mybir as mybir  # noqa: E402
from concourse import bacc, tile  # noqa: E402
from concourse.bass import ts  # noqa: E402
from concourse.bass_utils import run_bass_kernel_spmd  # noqa: E402

F32 = mybir.dt.float32
BF16 = mybir.dt.bfloat16
AF = mybir.ActivationFunctionType
ALU = mybir.AluOpType


def build_kernel():
    nc = bacc.Bacc(
        "TRN2",
        target_bir_lowering=False,
        debug=False,
        enable_asserts=False,
        num_devices=N_CORES,
    )
    xt = nc.dram_tensor("xt", [D, T], F32, kind="ExternalInput").ap()
    w1t = nc.dram_tensor("w1t", [D, H], F32, kind="ExternalInput").ap()
    b1 = nc.dram_tensor("b1", [H], F32, kind="ExternalInput").ap()
    w2t = nc.dram_tensor("w2t", [H, D], F32, kind="ExternalInput").ap()
    b2 = nc.dram_tensor("b2", [D], F32, kind="ExternalInput").ap()
    out = nc.dram_tensor("out", [D, T], F32, kind="ExternalOutput").ap()

    w1t_3d = w1t.rearrange("(k p) h -> k p h", p=128)   # [KD, 128, H]
    w2t_3d = w2t.rearrange("(k p) d -> k p d", p=128)   # [KH, 128, D]
    xt_3d = xt.rearrange("(k p) t -> p k t", p=128)     # [128, KD, T]
    out_3d = out.rearrange("(m p) t -> m p t", p=128)   # [KD, 128, T]

    with tile.TileContext(nc) as tc:
        with (
            tc.tile_pool(name="wb", bufs=1) as wbpool,
            tc.tile_pool(name="consts", bufs=1) as cpool,
        ):
            # persistent binarized weights (bf16 +-1)
            w1b = wbpool.tile([128, KD, H], BF16, tag="w1b")
            w2b = wbpool.tile([128, KH, D], BF16, tag="w2b")
            # per-output-channel constants, partition-major per 128-tile
            alpha1c = cpool.tile([128, KH], F32, tag="a1")
            alpha2c = cpool.tile([128, KD], F32, tag="a2")
            b1c = cpool.tile([128, KH], F32, tag="b1")
            b2c = cpool.tile([128, KD], F32, tag="b2")
            ones1 = cpool.tile([128, 1], BF16, tag="ones1")
            ones2 = cpool.tile([128, 1], BF16, tag="ones2")

            nc.vector.memset(ones1[:], 1.0 / D)
            nc.vector.memset(ones2[:], 1.0 / H)
            nc.sync.dma_start(b1c[:], b1.rearrange("(m p) -> p m", p=128))
            nc.sync.dma_start(b2c[:], b2.rearrange("(m p) -> p m", p=128))

            # ---- weight prep: binarize weights, alpha = mean|w| via PE ----
            # Weights are staged through a casting DMA (f32 DRAM -> bf16 SBUF):
            # sign(bf16(w)) == sign(w) and |bf16(w)| == bf16(|w|) exactly, and
            # bf16 stationaries keep LDWEIGHTS on the fast (FWL) path — the
            # fp32 weight path costs 2 passes at ~3x the LDW time.
            def w_prep(scope, wt_3d, ktiles, width, wb, aps, ones, mtiles, alphac,
                       spool, apool):
                with nc.named_scope(scope):
                    for k in range(ktiles):
                        st = spool.tile([128, H], BF16, tag="wstage", name=f"st_{scope}")
                        nc.gpsimd.dma_start(st[:, :width], wt_3d[k])
                        nc.scalar.activation(wb[:, k, :], st[:, :width], AF.Sign)
                        ab = apool.tile([128, H], BF16, tag="absw", name=f"ab_{scope}")
                        nc.vector.tensor_scalar(
                            ab[:, :width].bitcast(mybir.dt.uint16),
                            st[:, :width].bitcast(mybir.dt.uint16),
                            0x7FFF,
                            None,
                            ALU.bitwise_and,
                        )
                        for m in range(mtiles):
                            # start=True clears has_written for the WHOLE bank,
                            # so only the very first matmul into this psum tile
                            # may set it; later columns overwrite-on-first-touch
                            # via the per-element has_written bits.
                            nc.tensor.matmul(
                                aps[:, m : m + 1],
                                lhsT=ab[:, ts(m, 128)],
                                rhs=ones[:],
                                start=(k == 0 and m == 0),
                                stop=(k == ktiles - 1 and m == mtiles - 1),
                                skip_group_check=True,
                            )
                    nc.vector.tensor_copy(out=alphac[:], in_=aps[:])

            # ---- main: 4 token chunks, fc1 -> gelu -> fc2 ----
            with (
                tc.tile_pool(name="stage", bufs=2) as spool,
                tc.tile_pool(name="absw", bufs=2) as apool,
                tc.tile_pool(name="psa", bufs=2, space="PSUM") as psa,
                tc.tile_pool(name="xc", bufs=2) as xpool,
                tc.tile_pool(name="ht", bufs=1) as hpool,
                tc.tile_pool(name="oc", bufs=3) as opool,
                tc.tile_pool(name="ps1", bufs=3, space="PSUM") as ps1pool,
                tc.tile_pool(name="ps2", bufs=3, space="PSUM") as ps2pool,
            ):
                a1ps = psa.tile([128, KH], F32, tag="aps")
                a2ps = psa.tile([128, KD], F32, tag="aps")
                w_prep("w1prep", w1t_3d, KD, H, w1b, a1ps, ones1, KH, alpha1c,
                       spool, apool)
                for c in range(N_CHUNK):
                    csl = slice(c * TC, (c + 1) * TC)
                    with nc.named_scope(f"fc1_c{c}"):
                        xc = xpool.tile([128, KD, TC], BF16, tag="xc")
                        nc.gpsimd.dma_start(xc[:], xt_3d[:, :, csl])
                        ht = hpool.tile([128, KH, TC], BF16, tag="ht")
                        for m in range(KH):
                            ps = ps1pool.tile([128, TC], F32, tag="ps1")
                            for k in range(KD):
                                nc.tensor.matmul(
                                    ps[:],
                                    lhsT=w1b[:, k, ts(m, 128)],
                                    rhs=xc[:, k, :],
                                    start=(k == 0),
                                    stop=(k == KD - 1),
                                )
                            nc.scalar.activation(
                                ht[:, m, :],
                                ps[:],
                                AF.Gelu,
                                bias=b1c[:, m : m + 1],
                                scale=alpha1c[:, m : m + 1],
                            )
                    if c == 0:
                        # w2 prep hides under fc1_c0's PE work: its DMA/DVE/ACT
                        # overlap fc1 matmuls, and its alpha matmuls slot in
                        # between fc1_c0 and fc2_c0 on the PE.
                        w_prep("w2prep", w2t_3d, KH, D, w2b, a2ps, ones2, KD,
                               alpha2c, spool, apool)
                    with nc.named_scope(f"fc2_c{c}"):
                        for md in range(KD):
                            ps2 = ps2pool.tile([128, TC], F32, tag="ps2")
                            for mh in range(KH):
                                nc.tensor.matmul(
                                    ps2[:],
                                    lhsT=w2b[:, mh, ts(md, 128)],
                                    rhs=ht[:, mh, :],
                                    start=(mh == 0),
                                    stop=(mh == KH - 1),
                                )
                            oc = opool.tile([128, TC], F32, tag="oc")
                            nc.scalar.activation(
                                oc[:],
                                ps2[:],
                                AF.Identity,
                                bias=b2c[:, md : md + 1],
                                scale=alpha2c[:, md : md + 1],
                            )
                            nc.sync.dma_start(out_3d[md][:, csl], oc[:])

    nc.compile()
    return nc


_NC_CACHE = None


def _get_nc():
    global _NC_CACHE
    if _NC_CACHE is None:
        _NC_CACHE = build_kernel()
    return _NC_CACHE


def kernel(x, w1, b1, w2, b2):
    assert x.shape == (B, S, D) and w1.shape == (H, D) and w2.shape == (D, H)
    nc = _get_nc()

    xt = np.ascontiguousarray(x.reshape(T_GLOBAL, D).T)      # [D, 12544]
    w1t = np.ascontiguousarray(w1.T)                          # [D, H]
    w2t = np.ascontiguousarray(w2.T)                          # [H, D]
    b1 = np.ascontiguousarray(b1, dtype=np.float32)
    b2 = np.ascontiguousarray(b2, dtype=np.float32)

    in_maps = [
        {
            "xt": np.ascontiguousarray(xt[:, i * T : (i + 1) * T]),
            "w1t": w1t,
            "b1": b1,
            "w2t": w2t,
            "b2": b2,
        }
        for i in range(N_CORES)
    ]

    trace = bool(int(os.environ.get("BIMLP_TRACE", "0")))
    res = run_bass_kernel_spmd(
        nc, in_maps, core_ids=list(range(N_CORES)), trace=trace
    )
    if trace:
        kernel.last_results = res

    outt = np.concatenate([res.results[i]["out"] for i in range(N_CORES)], axis=1)
    return np.ascontiguousarray(outt.T).reshape(B, S, D).astype(np.float32)


# revision 11
# speedup vs baseline: 1.3161x; 1.1182x over previous
"""Binarized-weight MLP (BiMlp, 1w32a adaptive scaling) on 8 TRN2 NeuronCores.

Reference math (per token row x_t of [12544, 1024]):
    bw1 = mean(|w1|,axis=1,keepdims) * sign(w1)        # [4096, 1024]
    h   = gelu(x @ bw1.T + b1)                         # exact (erf) gelu
    bw2 = mean(|w2|,axis=1,keepdims) * sign(w2)        # [1024, 4096]
    out = h @ bw2.T + b2

Strategy: pure data-parallel over the 12544 tokens (1568/core, no collectives;
weights replicated — they fit in SBUF). All compute in a transposed layout
(hT = [H, tokens]) so per-output-channel alpha/bias ride the partition dim and
fuse into a single ScalarE activation (out = gelu(alpha*psum + bias)).
Matmuls run bf16 (sign weights are exact +-1 in bf16) accumulating fp32 in
PSUM; binarization (sign / |w| / alpha = mean|w|) is done on-device.

Host side only reshapes/transposes/shards (layout, not math) and reassembles.
"""

import os
import sys
import types

import numpy as np

N_CORES = 8
B, S, D, H = 64, 196, 1024, 4096
T_GLOBAL = B * S            # 12544 tokens
T = T_GLOBAL // N_CORES     # 1568 tokens per core
N_CHUNK = 4
TC = T // N_CHUNK           # 392 token columns per matmul (<=512 psum bank)
KD = D // 128               # 8 k-tiles over D
KH = H // 128               # 32 k-tiles over H


def _install_ntff_hook():
    """This container image lacks antenv.axon_hooks; synthesize it so
    run_bass_kernel_spmd(trace=True) can capture NTFF profiles through the
    libaxon_pjrt C ABI (the same hook trn_boot would register)."""
    if "antenv.axon_hooks" in sys.modules:
        return
    import contextlib
    import ctypes

    try:
        lib = ctypes.CDLL("/opt/axon/libaxon_pjrt.so")
        lib.axon_start_nrt_profile.argtypes = [
            ctypes.POINTER(ctypes.c_int64),
            ctypes.c_size_t,
        ]
        lib.axon_start_nrt_profile.restype = ctypes.c_int64
        lib.axon_stop_nrt_profile.argtypes = [ctypes.c_char_p]
        lib.axon_stop_nrt_profile.restype = ctypes.c_int64
    except (OSError, AttributeError):
        return

    @contextlib.contextmanager
    def _hook(output_dir, device_ids):
        import jax

        jax.devices()
        if device_ids:
            ids = (ctypes.c_int64 * len(device_ids))(*device_ids)
            rc = lib.axon_start_nrt_profile(ids, len(device_ids))
        else:
            rc = lib.axon_start_nrt_profile(None, 0)
        if rc != 0:
            raise RuntimeError(f"axon_start_nrt_profile rc={rc}")
        try:
            yield
        finally:
            n = lib.axon_stop_nrt_profile(str(output_dir).encode())
            print(f"ntff profile: {n} file(s) in {output_dir}", file=sys.stderr)

    mod = types.ModuleType("antenv.axon_hooks")
    mod.get_axon_ntff_profile_hook = lambda: _hook
    mod.set_axon_ntff_profile_hook = lambda h: None
    sys.modules["antenv.axon_hooks"] = mod


_install_ntff_hook()

import concourse.mybir as mybir  # noqa: E402
from concourse import bacc, tile  # noqa: E402
from concourse.bass import ts  # noqa: E402
from concourse.bass_utils import run_bass_kernel_spmd  # noqa: E402

F32 = mybir.dt.float32
BF16 = mybir.dt.bfloat16
AF = mybir.ActivationFunctionType
ALU = mybir.AluOpType


def build_kernel():
    nc = bacc.Bacc(
        "TRN2",
        target_bir_lowering=False,
        debug=False,
        enable_asserts=False,
        num_devices=N_CORES,
    )
    xt = nc.dram_tensor("xt", [D, T], F32, kind="ExternalInput").ap()
    # weights ship as bf16: identical rounding to an on-device f32->bf16 cast,
    # half the wire bytes, and staging stays on the fast HWDGE (non-casting)
    # DMA path. sign/|.| of the bf16 value match sign/|.| of the f32 value
    # to bf16 precision, which is all the bf16 matmul consumes anyway.
    w1t = nc.dram_tensor("w1t", [D, H], BF16, kind="ExternalInput").ap()
    b1 = nc.dram_tensor("b1", [H], F32, kind="ExternalInput").ap()
    w2t = nc.dram_tensor("w2t", [H, D], BF16, kind="ExternalInput").ap()
    b2 = nc.dram_tensor("b2", [D], F32, kind="ExternalInput").ap()
    out = nc.dram_tensor("out", [D, T], F32, kind="ExternalOutput").ap()

    w1t_3d = w1t.rearrange("(k p) h -> k p h", p=128)   # [KD, 128, H]
    w2t_3d = w2t.rearrange("(k p) d -> k p d", p=128)   # [KH, 128, D]
    xt_3d = xt.rearrange("(k p) t -> p k t", p=128)     # [128, KD, T]
    out_3d = out.rearrange("(m p) t -> m p t", p=128)   # [KD, 128, T]

    with tile.TileContext(nc) as tc:
        with (
            tc.tile_pool(name="wb", bufs=1) as wbpool,
            tc.tile_pool(name="consts", bufs=1) as cpool,
        ):
            # persistent binarized weights (bf16 +-1)
            w1b = wbpool.tile([128, KD, H], BF16, tag="w1b")
            w2b = wbpool.tile([128, KH, D], BF16, tag="w2b")
            # per-output-channel constants, partition-major per 128-tile
            alpha1c = cpool.tile([128, KH], F32, tag="a1")
            alpha2c = cpool.tile([128, KD], F32, tag="a2")
            b1c = cpool.tile([128, KH], F32, tag="b1")
            b2c = cpool.tile([128, KD], F32, tag="b2")
            ones1 = cpool.tile([128, 1], BF16, tag="ones1")
            ones2 = cpool.tile([128, 1], BF16, tag="ones2")

            nc.vector.memset(ones1[:], 1.0 / D)
            nc.vector.memset(ones2[:], 1.0 / H)
            nc.sync.dma_start(b1c[:], b1.rearrange("(m p) -> p m", p=128))
            nc.sync.dma_start(b2c[:], b2.rearrange("(m p) -> p m", p=128))

            # ---- weight prep: binarize weights, alpha = mean|w| via PE ----
            # Weights are staged through a casting DMA (f32 DRAM -> bf16 SBUF):
            # sign(bf16(w)) == sign(w) and |bf16(w)| == bf16(|w|) exactly, and
            # bf16 stationaries keep LDWEIGHTS on the fast (FWL) path — the
            # fp32 weight path costs 2 passes at ~3x the LDW time.
            def w_prep(scope, wt_3d, ktiles, width, wb, aps, ones, mtiles, alphac,
                       spool, apool):
                with nc.named_scope(scope):
                    for k in range(ktiles):
                        st = spool.tile([128, H], BF16, tag="wstage", name=f"st_{scope}")
                        nc.sync.dma_start(st[:, :width], wt_3d[k])
                        nc.scalar.activation(wb[:, k, :], st[:, :width], AF.Sign)
                        ab = apool.tile([128, H], BF16, tag="absw", name=f"ab_{scope}")
                        nc.vector.tensor_scalar(
                            ab[:, :width].bitcast(mybir.dt.uint16),
                            st[:, :width].bitcast(mybir.dt.uint16),
                            0x7FFF,
                            None,
                            ALU.bitwise_and,
                        )
                        for m in range(mtiles):
                            # start=True clears has_written for the WHOLE bank,
                            # so only the very first matmul into this psum tile
                            # may set it; later columns overwrite-on-first-touch
                            # via the per-element has_written bits.
                            nc.tensor.matmul(
                                aps[:, m : m + 1],
                                lhsT=ab[:, ts(m, 128)],
                                rhs=ones[:],
                                start=(k == 0 and m == 0),
                                stop=(k == ktiles - 1 and m == mtiles - 1),
                                skip_group_check=True,
                            )
                    nc.vector.tensor_copy(out=alphac[:], in_=aps[:])

            # ---- main: 4 token chunks, fc1 -> gelu -> fc2 ----
            with (
                tc.tile_pool(name="stage", bufs=2) as spool,
                tc.tile_pool(name="absw", bufs=2) as apool,
                tc.tile_pool(name="psa", bufs=2, space="PSUM") as psa,
                tc.tile_pool(name="xc", bufs=2) as xpool,
                tc.tile_pool(name="ht", bufs=1) as hpool,
                tc.tile_pool(name="oc", bufs=3) as opool,
                tc.tile_pool(name="ps1", bufs=3, space="PSUM") as ps1pool,
                tc.tile_pool(name="ps2", bufs=3, space="PSUM") as ps2pool,
            ):
                a1ps = psa.tile([128, KH], F32, tag="aps")
                a2ps = psa.tile([128, KD], F32, tag="aps")
                w_prep("w1prep", w1t_3d, KD, H, w1b, a1ps, ones1, KH, alpha1c,
                       spool, apool)
                for c in range(N_CHUNK):
                    csl = slice(c * TC, (c + 1) * TC)
                    with nc.named_scope(f"fc1_c{c}"):
                        xc = xpool.tile([128, KD, TC], BF16, tag="xc")
                        nc.gpsimd.dma_start(xc[:], xt_3d[:, :, csl])
                        ht = hpool.tile([128, KH, TC], BF16, tag="ht")
                        for m in range(KH):
                            ps = ps1pool.tile([128, TC], F32, tag="ps1")
                            for k in range(KD):
                                nc.tensor.matmul(
                                    ps[:],
                                    lhsT=w1b[:, k, ts(m, 128)],
                                    rhs=xc[:, k, :],
                                    start=(k == 0),
                                    stop=(k == KD - 1),
                                )
                            nc.scalar.activation(
                                ht[:, m, :],
                                ps[:],
                                AF.Gelu,
                                bias=b1c[:, m : m + 1],
                                scale=alpha1c[:, m : m + 1],
                            )
                    if c == 0:
                        # w2 prep hides under fc1_c0's PE work: its DMA/DVE/ACT
                        # overlap fc1 matmuls, and its alpha matmuls slot in
                        # between fc1_c0 and fc2_c0 on the PE.
                        w_prep("w2prep", w2t_3d, KH, D, w2b, a2ps, ones2, KD,
                               alpha2c, spool, apool)
                    with nc.named_scope(f"fc2_c{c}"):
                        for md in range(KD):
                            ps2 = ps2pool.tile([128, TC], F32, tag="ps2")
                            for mh in range(KH):
                                nc.tensor.matmul(
                                    ps2[:],
                                    lhsT=w2b[:, mh, ts(md, 128)],
                                    rhs=ht[:, mh, :],
                                    start=(mh == 0),
                                    stop=(mh == KH - 1),
                                )
                            oc = opool.tile([128, TC], F32, tag="oc")
                            nc.scalar.activation(
                                oc[:],
                                ps2[:],
                                AF.Identity,
                                bias=b2c[:, md : md + 1],
                                scale=alpha2c[:, md : md + 1],
                            )
                            nc.sync.dma_start(out_3d[md][:, csl], oc[:])

    nc.compile()
    return nc


_NC_CACHE = None


def _get_nc():
    global _NC_CACHE
    if _NC_CACHE is None:
        _NC_CACHE = build_kernel()
    return _NC_CACHE


def kernel(x, w1, b1, w2, b2):
    assert x.shape == (B, S, D) and w1.shape == (H, D) and w2.shape == (D, H)
    nc = _get_nc()

    import ml_dtypes

    xt = np.ascontiguousarray(x.reshape(T_GLOBAL, D).T)      # [D, 12544]
    w1t = np.ascontiguousarray(w1.T).astype(ml_dtypes.bfloat16)   # [D, H]
    w2t = np.ascontiguousarray(w2.T).astype(ml_dtypes.bfloat16)   # [H, D]
    b1 = np.ascontiguousarray(b1, dtype=np.float32)
    b2 = np.ascontiguousarray(b2, dtype=np.float32)

    in_maps = [
        {
            "xt": np.ascontiguousarray(xt[:, i * T : (i + 1) * T]),
            "w1t": w1t,
            "b1": b1,
            "w2t": w2t,
            "b2": b2,
        }
        for i in range(N_CORES)
    ]

    trace = bool(int(os.environ.get("BIMLP_TRACE", "0")))
    res = run_bass_kernel_spmd(
        nc, in_maps, core_ids=list(range(N_CORES)), trace=trace
    )
    if trace:
        kernel.last_results = res

    outt = np.concatenate([res.results[i]["out"] for i in range(N_CORES)], axis=1)
    return np.ascontiguousarray(outt.T).reshape(B, S, D).astype(np.float32)


# revision 15
# speedup vs baseline: 1.3739x; 1.0439x over previous
"""Binarized-weight MLP (BiMlp, 1w32a adaptive scaling) on 8 TRN2 NeuronCores.

Reference math (per token row x_t of [12544, 1024]):
    bw1 = mean(|w1|,axis=1,keepdims) * sign(w1)        # [4096, 1024]
    h   = gelu(x @ bw1.T + b1)                         # exact (erf) gelu
    bw2 = mean(|w2|,axis=1,keepdims) * sign(w2)        # [1024, 4096]
    out = h @ bw2.T + b2

Strategy: pure data-parallel over the 12544 tokens (1568/core, no collectives;
weights replicated — they fit in SBUF). All compute in a transposed layout
(hT = [H, tokens]) so per-output-channel alpha/bias ride the partition dim and
fuse into a single ScalarE activation (out = gelu(alpha*psum + bias)).
Matmuls run bf16 (sign weights are exact +-1 in bf16) accumulating fp32 in
PSUM; binarization (sign / |w| / alpha = mean|w|) is done on-device.

Host side only reshapes/transposes/shards (layout, not math) and reassembles.
"""

import os
import sys
import types

import numpy as np

N_CORES = 8
B, S, D, H = 64, 196, 1024, 4096
T_GLOBAL = B * S            # 12544 tokens
T = T_GLOBAL // N_CORES     # 1568 tokens per core
N_CHUNK = 4
TC = T // N_CHUNK           # 392 token columns per matmul (<=512 psum bank)
KD = D // 128               # 8 k-tiles over D
KH = H // 128               # 32 k-tiles over H


def _install_ntff_hook():
    """This container image lacks antenv.axon_hooks; synthesize it so
    run_bass_kernel_spmd(trace=True) can capture NTFF profiles through the
    libaxon_pjrt C ABI (the same hook trn_boot would register)."""
    if "antenv.axon_hooks" in sys.modules:
        return
    import contextlib
    import ctypes

    try:
        lib = ctypes.CDLL("/opt/axon/libaxon_pjrt.so")
        lib.axon_start_nrt_profile.argtypes = [
            ctypes.POINTER(ctypes.c_int64),
            ctypes.c_size_t,
        ]
        lib.axon_start_nrt_profile.restype = ctypes.c_int64
        lib.axon_stop_nrt_profile.argtypes = [ctypes.c_char_p]
        lib.axon_stop_nrt_profile.restype = ctypes.c_int64
    except (OSError, AttributeError):
        return

    @contextlib.contextmanager
    def _hook(output_dir, device_ids):
        import jax

        jax.devices()
        if device_ids:
            ids = (ctypes.c_int64 * len(device_ids))(*device_ids)
            rc = lib.axon_start_nrt_profile(ids, len(device_ids))
        else:
            rc = lib.axon_start_nrt_profile(None, 0)
        if rc != 0:
            raise RuntimeError(f"axon_start_nrt_profile rc={rc}")
        try:
            yield
        finally:
            n = lib.axon_stop_nrt_profile(str(output_dir).encode())
            print(f"ntff profile: {n} file(s) in {output_dir}", file=sys.stderr)

    mod = types.ModuleType("antenv.axon_hooks")
    mod.get_axon_ntff_profile_hook = lambda: _hook
    mod.set_axon_ntff_profile_hook = lambda h: None
    sys.modules["antenv.axon_hooks"] = mod


_install_ntff_hook()

import concourse.mybir as mybir  # noqa: E402
from concourse import bacc, tile  # noqa: E402
from concourse.bass import ts  # noqa: E402
from concourse.bass_utils import run_bass_kernel_spmd  # noqa: E402

F32 = mybir.dt.float32
BF16 = mybir.dt.bfloat16
AF = mybir.ActivationFunctionType
ALU = mybir.AluOpType


def build_kernel():
    nc = bacc.Bacc(
        "TRN2",
        target_bir_lowering=False,
        debug=False,
        enable_asserts=False,
        num_devices=N_CORES,
    )
    xt = nc.dram_tensor("xt", [D, T], F32, kind="ExternalInput").ap()
    # weights ship as bf16: identical rounding to an on-device f32->bf16 cast,
    # half the wire bytes, and staging stays on the fast HWDGE (non-casting)
    # DMA path. sign/|.| of the bf16 value match sign/|.| of the f32 value
    # to bf16 precision, which is all the bf16 matmul consumes anyway.
    w1t = nc.dram_tensor("w1t", [D, H], BF16, kind="ExternalInput").ap()
    b1 = nc.dram_tensor("b1", [H], F32, kind="ExternalInput").ap()
    w2t = nc.dram_tensor("w2t", [H, D], BF16, kind="ExternalInput").ap()
    b2 = nc.dram_tensor("b2", [D], F32, kind="ExternalInput").ap()
    out = nc.dram_tensor("out", [D, T], F32, kind="ExternalOutput").ap()

    w1t_3d = w1t.rearrange("(k p) h -> k p h", p=128)   # [KD, 128, H]
    w2t_3d = w2t.rearrange("(k p) d -> k p d", p=128)   # [KH, 128, D]
    xt_3d = xt.rearrange("(k p) t -> p k t", p=128)     # [128, KD, T]
    out_3d = out.rearrange("(m p) t -> m p t", p=128)   # [KD, 128, T]

    with tile.TileContext(nc) as tc:
        with (
            tc.tile_pool(name="wb", bufs=1) as wbpool,
            tc.tile_pool(name="consts", bufs=1) as cpool,
        ):
            # persistent binarized weights (bf16 +-1)
            w1b = wbpool.tile([128, KD, H], BF16, tag="w1b")
            w2b = wbpool.tile([128, KH, D], BF16, tag="w2b")
            # per-output-channel constants, partition-major per 128-tile
            alpha1c = cpool.tile([128, KH], F32, tag="a1")
            alpha2c = cpool.tile([128, KD], F32, tag="a2")
            b1c = cpool.tile([128, KH], F32, tag="b1")
            b2c = cpool.tile([128, KD], F32, tag="b2")
            ones1 = cpool.tile([128, 1], BF16, tag="ones1")
            ones2 = cpool.tile([128, 1], BF16, tag="ones2")

            nc.vector.memset(ones1[:], 1.0 / D)
            nc.vector.memset(ones2[:], 1.0 / H)
            nc.sync.dma_start(b1c[:], b1.rearrange("(m p) -> p m", p=128))
            nc.sync.dma_start(b2c[:], b2.rearrange("(m p) -> p m", p=128))

            dpool = tc.alloc_tile_pool(name="adram", bufs=1, space="DRAM")
            a1d = dpool.tile([H], F32, tag="a1d")
            a2d = dpool.tile([D], F32, tag="a2d")

            # ---- w1 prep ----
            # alpha row = (1/D * ones)^T @ |w1t| with ones STATIONARY: the
            # 1-column LDWEIGHTS is free and the N=512 moving operand keeps the
            # PE array dense (N=1 matmuls starve the HAM activity monitor and
            # re-throttle the clock to 1.2GHz). The [1, H] row accumulates
            # per-bank (each 512-chunk is exactly one PSUM bank), then bounces
            # through DRAM to become the per-partition column layout.
            with (
                tc.tile_pool(name="w1stage", bufs=4) as s1pool,
                tc.tile_pool(name="absw1", bufs=2) as a1pool,
                tc.tile_pool(name="row1", bufs=1) as r1pool,
                tc.tile_pool(name="psrow1", bufs=1, space="PSUM") as pr1,
            ):
                with nc.named_scope("w1prep"):
                    a1row_ps = pr1.tile([1, H], F32, tag="a1row")
                    for k in range(KD):
                        st = s1pool.tile([128, H], BF16, tag="w1stage")
                        nc.sync.dma_start(st[:], w1t_3d[k])
                        nc.scalar.activation(w1b[:, k, :], st[:], AF.Sign)
                        ab = a1pool.tile([128, H], BF16, tag="absw1")
                        nc.vector.tensor_scalar(
                            ab[:].bitcast(mybir.dt.uint16),
                            st[:].bitcast(mybir.dt.uint16),
                            0x7FFF,
                            None,
                            ALU.bitwise_and,
                        )
                        for n in range(H // 512):
                            nc.tensor.matmul(
                                a1row_ps[:, ts(n, 512)],
                                lhsT=ones1[:],
                                rhs=ab[:, ts(n, 512)],
                                start=(k == 0),
                                stop=(k == KD - 1),
                                skip_group_check=True,
                            )
                    a1row = r1pool.tile([1, H], F32, tag="a1row_sb")
                    nc.vector.tensor_copy(out=a1row[:], in_=a1row_ps[:])
                    nc.sync.dma_start(a1d[:], a1row[:])
                    nc.sync.dma_start(
                        alpha1c[:], a1d.rearrange("(m p) -> p m", p=128)
                    )

            # ---- main: 4 token chunks, fc1 -> gelu -> fc2 ----
            with (
                tc.tile_pool(name="absw2", bufs=16) as a2pool,
                tc.tile_pool(name="row2", bufs=1) as r2pool,
                tc.tile_pool(name="xc", bufs=2) as xpool,
                tc.tile_pool(name="ht", bufs=1) as hpool,
                tc.tile_pool(name="oc", bufs=3) as opool,
                tc.tile_pool(name="psrow2", bufs=1, space="PSUM") as pr2,
                tc.tile_pool(name="ps1", bufs=4, space="PSUM") as ps1pool,
                tc.tile_pool(name="ps2", bufs=2, space="PSUM") as ps2pool,
            ):
                # w2 lands directly in its persistent SBUF tile (no staging
                # slots to recycle): DMA raw -> abs to scratch -> sign in
                # place. DMAs and the first 16 abs/sign run under fc1_c0.
                a2row_ps = pr2.tile([1, D], F32, tag="a2row")

                def w2_dma_all():
                    for k in range(KH):
                        nc.sync.dma_start(w2b[:, k, :], w2t_3d[k])

                def w2_absign(k):
                    ab2 = a2pool.tile([128, D], BF16, tag="absw2",
                                      name=f"ab2_{k}")
                    nc.vector.tensor_scalar(
                        ab2[:].bitcast(mybir.dt.uint16),
                        w2b[:, k, :].bitcast(mybir.dt.uint16),
                        0x7FFF,
                        None,
                        ALU.bitwise_and,
                    )
                    # sign on DVE (ScalarE is loaded with gelus + w1 signs):
                    # bf16 sign(w) == (w & 0x8000) | 0x3f80, in place, ordered
                    # after the abs read by same-engine program order.
                    nc.vector.tensor_scalar(
                        w2b[:, k, :].bitcast(mybir.dt.uint16),
                        w2b[:, k, :].bitcast(mybir.dt.uint16),
                        0x8000,
                        0x3F80,
                        ALU.bitwise_and,
                        ALU.bitwise_or,
                    )
                    return ab2

                w2_dma_all()
                ab2_tiles = {}
                for k in range(16):
                    ab2_tiles[k] = w2_absign(k)

                for c in range(N_CHUNK):
                    csl = slice(c * TC, (c + 1) * TC)
                    with nc.named_scope(f"fc1_c{c}"):
                        xc = xpool.tile([128, KD, TC], BF16, tag="xc")
                        nc.gpsimd.dma_start(xc[:], xt_3d[:, :, csl])
                        ht = hpool.tile([128, KH, TC], BF16, tag="ht")
                        for m in range(KH):
                            ps = ps1pool.tile([128, TC], F32, tag="ps1")
                            for k in range(KD):
                                nc.tensor.matmul(
                                    ps[:],
                                    lhsT=w1b[:, k, ts(m, 128)],
                                    rhs=xc[:, k, :],
                                    start=(k == 0),
                                    stop=(k == KD - 1),
                                )
                            nc.scalar.activation(
                                ht[:, m, :],
                                ps[:],
                                AF.Gelu,
                                bias=b1c[:, m : m + 1],
                                scale=alpha1c[:, m : m + 1],
                            )
                    if c == 0:
                        # alpha2: ones-stationary matmuls over |w2| slot in
                        # between fc1_c0 and fc2_c0 on the PE; the late abs
                        # tiles recycle the 16 scratch slots as they drain.
                        with nc.named_scope("w2prep"):
                            for k in range(KH):
                                ab2 = ab2_tiles.pop(k, None)
                                if ab2 is None:
                                    ab2 = w2_absign(k)
                                for n in range(D // 512):
                                    nc.tensor.matmul(
                                        a2row_ps[:, ts(n, 512)],
                                        lhsT=ones2[:],
                                        rhs=ab2[:, ts(n, 512)],
                                        start=(k == 0),
                                        stop=(k == KH - 1),
                                        skip_group_check=True,
                                    )
                            a2row = r2pool.tile([1, D], F32, tag="a2row_sb")
                            nc.vector.tensor_copy(out=a2row[:], in_=a2row_ps[:])
                            nc.sync.dma_start(a2d[:], a2row[:])
                            nc.sync.dma_start(
                                alpha2c[:], a2d.rearrange("(m p) -> p m", p=128)
                            )
                    with nc.named_scope(f"fc2_c{c}"):
                        for md in range(KD):
                            ps2 = ps2pool.tile([128, TC], F32, tag="ps2")
                            for mh in range(KH):
                                nc.tensor.matmul(
                                    ps2[:],
                                    lhsT=w2b[:, mh, ts(md, 128)],
                                    rhs=ht[:, mh, :],
                                    start=(mh == 0),
                                    stop=(mh == KH - 1),
                                )
                            oc = opool.tile([128, TC], F32, tag="oc")
                            nc.scalar.activation(
                                oc[:],
                                ps2[:],
                                AF.Identity,
                                bias=b2c[:, md : md + 1],
                                scale=alpha2c[:, md : md + 1],
                            )
                            nc.sync.dma_start(out_3d[md][:, csl], oc[:])

    nc.compile()
    return nc


_NC_CACHE = None


def _get_nc():
    global _NC_CACHE
    if _NC_CACHE is None:
        _NC_CACHE = build_kernel()
    return _NC_CACHE


def kernel(x, w1, b1, w2, b2):
    assert x.shape == (B, S, D) and w1.shape == (H, D) and w2.shape == (D, H)
    nc = _get_nc()

    import ml_dtypes

    xt = np.ascontiguousarray(x.reshape(T_GLOBAL, D).T)      # [D, 12544]
    w1t = np.ascontiguousarray(w1.T).astype(ml_dtypes.bfloat16)   # [D, H]
    w2t = np.ascontiguousarray(w2.T).astype(ml_dtypes.bfloat16)   # [H, D]
    b1 = np.ascontiguousarray(b1, dtype=np.float32)
    b2 = np.ascontiguousarray(b2, dtype=np.float32)

    in_maps = [
        {
            "xt": np.ascontiguousarray(xt[:, i * T : (i + 1) * T]),
            "w1t": w1t,
            "b1": b1,
            "w2t": w2t,
            "b2": b2,
        }
        for i in range(N_CORES)
    ]

    trace = bool(int(os.environ.get("BIMLP_TRACE", "0")))
    res = run_bass_kernel_spmd(
        nc, in_maps, core_ids=list(range(N_CORES)), trace=trace
    )
    if trace:
        kernel.last_results = res

    outt = np.concatenate([res.results[i]["out"] for i in range(N_CORES)], axis=1)
    return np.ascontiguousarray(outt.T).reshape(B, S, D).astype(np.float32)


# revision 19
# speedup vs baseline: 1.4136x; 1.0289x over previous
"""Binarized-weight MLP (BiMlp, 1w32a adaptive scaling) on 8 TRN2 NeuronCores.

Reference math (per token row x_t of [12544, 1024]):
    bw1 = mean(|w1|,axis=1,keepdims) * sign(w1)        # [4096, 1024]
    h   = gelu(x @ bw1.T + b1)                         # exact (erf) gelu
    bw2 = mean(|w2|,axis=1,keepdims) * sign(w2)        # [1024, 4096]
    out = h @ bw2.T + b2

Strategy: pure data-parallel over the 12544 tokens (1568/core, no collectives;
weights replicated — they fit in SBUF). All compute in a transposed layout
(hT = [H, tokens]) so per-output-channel alpha/bias ride the partition dim and
fuse into a single ScalarE activation (out = gelu(alpha*psum + bias)).
Matmuls run bf16 (sign weights are exact +-1 in bf16) accumulating fp32 in
PSUM; binarization (sign / |w| / alpha = mean|w|) is done on-device.

Host side only reshapes/transposes/shards (layout, not math) and reassembles.
"""

import os
import sys
import types

import numpy as np

N_CORES = 8
B, S, D, H = 64, 196, 1024, 4096
T_GLOBAL = B * S            # 12544 tokens
T = T_GLOBAL // N_CORES     # 1568 tokens per core
N_CHUNK = 4
TC = T // N_CHUNK           # 392 token columns per matmul (<=512 psum bank)
KD = D // 128               # 8 k-tiles over D
KH = H // 128               # 32 k-tiles over H


def _install_ntff_hook():
    """This container image lacks antenv.axon_hooks; synthesize it so
    run_bass_kernel_spmd(trace=True) can capture NTFF profiles through the
    libaxon_pjrt C ABI (the same hook trn_boot would register)."""
    if "antenv.axon_hooks" in sys.modules:
        return
    import contextlib
    import ctypes

    try:
        lib = ctypes.CDLL("/opt/axon/libaxon_pjrt.so")
        lib.axon_start_nrt_profile.argtypes = [
            ctypes.POINTER(ctypes.c_int64),
            ctypes.c_size_t,
        ]
        lib.axon_start_nrt_profile.restype = ctypes.c_int64
        lib.axon_stop_nrt_profile.argtypes = [ctypes.c_char_p]
        lib.axon_stop_nrt_profile.restype = ctypes.c_int64
    except (OSError, AttributeError):
        return

    @contextlib.contextmanager
    def _hook(output_dir, device_ids):
        import jax

        jax.devices()
        if device_ids:
            ids = (ctypes.c_int64 * len(device_ids))(*device_ids)
            rc = lib.axon_start_nrt_profile(ids, len(device_ids))
        else:
            rc = lib.axon_start_nrt_profile(None, 0)
        if rc != 0:
            raise RuntimeError(f"axon_start_nrt_profile rc={rc}")
        try:
            yield
        finally:
            n = lib.axon_stop_nrt_profile(str(output_dir).encode())
            print(f"ntff profile: {n} file(s) in {output_dir}", file=sys.stderr)

    mod = types.ModuleType("antenv.axon_hooks")
    mod.get_axon_ntff_profile_hook = lambda: _hook
    mod.set_axon_ntff_profile_hook = lambda h: None
    sys.modules["antenv.axon_hooks"] = mod


_install_ntff_hook()

import concourse.mybir as mybir  # noqa: E402
from concourse import bacc, tile  # noqa: E402
from concourse.bass import ts  # noqa: E402
from concourse.bass_utils import run_bass_kernel_spmd  # noqa: E402

F32 = mybir.dt.float32
BF16 = mybir.dt.bfloat16
AF = mybir.ActivationFunctionType
ALU = mybir.AluOpType


def build_kernel():
    nc = bacc.Bacc(
        "TRN2",
        target_bir_lowering=False,
        debug=False,
        enable_asserts=False,
        num_devices=N_CORES,
    )
    # x ships bf16 for the same reason as the weights: the bf16 matmul
    # consumes bf16(x) either way; host-side cast == the casting DMA it
    # replaces, but rides the fast HWDGE path at half the wire bytes.
    xt = nc.dram_tensor("xt", [D, T], BF16, kind="ExternalInput").ap()
    # weights ship as bf16: identical rounding to an on-device f32->bf16 cast,
    # half the wire bytes, and staging stays on the fast HWDGE (non-casting)
    # DMA path. sign/|.| of the bf16 value match sign/|.| of the f32 value
    # to bf16 precision, which is all the bf16 matmul consumes anyway.
    w1t = nc.dram_tensor("w1t", [D, H], BF16, kind="ExternalInput").ap()
    b1 = nc.dram_tensor("b1", [H], F32, kind="ExternalInput").ap()
    w2t = nc.dram_tensor("w2t", [H, D], BF16, kind="ExternalInput").ap()
    b2 = nc.dram_tensor("b2", [D], F32, kind="ExternalInput").ap()
    out = nc.dram_tensor("out", [D, T], F32, kind="ExternalOutput").ap()

    w1t_3d = w1t.rearrange("(k p) h -> k p h", p=128)   # [KD, 128, H]
    w2t_3d = w2t.rearrange("(k p) d -> k p d", p=128)   # [KH, 128, D]
    xt_3d = xt.rearrange("(k p) t -> p k t", p=128)     # [128, KD, T]
    out_3d = out.rearrange("(m p) t -> m p t", p=128)   # [KD, 128, T]

    with tile.TileContext(nc) as tc:
        with (
            tc.tile_pool(name="wb", bufs=1) as wbpool,
            tc.tile_pool(name="consts", bufs=1) as cpool,
        ):
            # persistent binarized weights (bf16 +-1)
            w1b = wbpool.tile([128, KD, H], BF16, tag="w1b")
            w2b = wbpool.tile([128, KH, D], BF16, tag="w2b")
            # per-output-channel constants, partition-major per 128-tile
            alpha1c = cpool.tile([128, KH], F32, tag="a1")
            alpha2c = cpool.tile([128, KD], F32, tag="a2")
            b1c = cpool.tile([128, KH], F32, tag="b1")
            b2c = cpool.tile([128, KD], F32, tag="b2")
            ones1 = cpool.tile([128, 1], BF16, tag="ones1")
            ones2 = cpool.tile([128, 1], BF16, tag="ones2")

            nc.vector.memset(ones1[:], 1.0 / D)
            nc.vector.memset(ones2[:], 1.0 / H)
            nc.sync.dma_start(b1c[:], b1.rearrange("(m p) -> p m", p=128))
            nc.sync.dma_start(b2c[:], b2.rearrange("(m p) -> p m", p=128))

            dpool = tc.alloc_tile_pool(name="adram", bufs=1, space="DRAM")
            a1d = dpool.tile([H], F32, tag="a1d")
            a2d = dpool.tile([D], F32, tag="a2d")

            # ---- w1 prep ----
            # alpha row = (1/D * ones)^T @ |w1t| with ones STATIONARY: the
            # 1-column LDWEIGHTS is free and the N=512 moving operand keeps the
            # PE array dense (N=1 matmuls starve the HAM activity monitor and
            # re-throttle the clock to 1.2GHz). The [1, H] row accumulates
            # per-bank (each 512-chunk is exactly one PSUM bank), then bounces
            # through DRAM to become the per-partition column layout.
            with (
                tc.tile_pool(name="w1stage", bufs=4) as s1pool,
                tc.tile_pool(name="absw1", bufs=2) as a1pool,
                tc.tile_pool(name="row1", bufs=1) as r1pool,
                tc.tile_pool(name="psrow1", bufs=1, space="PSUM") as pr1,
            ):
                with nc.named_scope("w1prep"):
                    a1row_ps = pr1.tile([1, H], F32, tag="a1row")
                    # prime the HAM clock gate: ~20 dense throwaway matmuls
                    # while the weight DMA streams in, so the 2.4GHz clock is
                    # already up when real work lands (results overwritten by
                    # the k==0 start=True below).
                    warm = cpool.tile([128, 512], BF16, tag="warm")
                    nc.vector.memset(warm[:], 1.0)
                    for _ in range(20):
                        nc.tensor.matmul(
                            a1row_ps[:, 0:512],
                            lhsT=ones1[:],
                            rhs=warm[:],
                            start=True,
                            stop=True,
                            skip_group_check=True,
                        )
                    for k in range(KD):
                        st = s1pool.tile([128, H], BF16, tag="w1stage")
                        nc.sync.dma_start(st[:], w1t_3d[k])
                        if k < 5:
                            nc.scalar.activation(w1b[:, k, :], st[:], AF.Sign)
                        else:
                            # DVE bit-trick sign to cut the serial ScalarE
                            # chain that gates fc1's first matmul
                            nc.vector.tensor_scalar(
                                w1b[:, k, :].bitcast(mybir.dt.uint16),
                                st[:].bitcast(mybir.dt.uint16),
                                0x8000,
                                0x3F80,
                                ALU.bitwise_and,
                                ALU.bitwise_or,
                            )
                        ab = a1pool.tile([128, H], BF16, tag="absw1")
                        nc.vector.tensor_scalar(
                            ab[:].bitcast(mybir.dt.uint16),
                            st[:].bitcast(mybir.dt.uint16),
                            0x7FFF,
                            None,
                            ALU.bitwise_and,
                        )
                        for n in range(H // 512):
                            nc.tensor.matmul(
                                a1row_ps[:, ts(n, 512)],
                                lhsT=ones1[:],
                                rhs=ab[:, ts(n, 512)],
                                start=(k == 0),
                                stop=(k == KD - 1),
                                skip_group_check=True,
                            )
                    a1row = r1pool.tile([1, H], F32, tag="a1row_sb")
                    nc.vector.tensor_copy(out=a1row[:], in_=a1row_ps[:])
                    nc.sync.dma_start(a1d[:], a1row[:])
                    nc.sync.dma_start(
                        alpha1c[:], a1d.rearrange("(m p) -> p m", p=128)
                    )

            # ---- main: 4 token chunks, fc1 -> gelu -> fc2 ----
            with (
                tc.tile_pool(name="absw2", bufs=16) as a2pool,
                tc.tile_pool(name="row2", bufs=1) as r2pool,
                tc.tile_pool(name="xc", bufs=2) as xpool,
                tc.tile_pool(name="ht", bufs=1) as hpool,
                tc.tile_pool(name="oc", bufs=3) as opool,
                tc.tile_pool(name="psrow2", bufs=1, space="PSUM") as pr2,
                tc.tile_pool(name="ps1", bufs=4, space="PSUM") as ps1pool,
                tc.tile_pool(name="ps2", bufs=2, space="PSUM") as ps2pool,
            ):
                # w2 lands directly in its persistent SBUF tile (no staging
                # slots to recycle): DMA raw -> abs to scratch -> sign in
                # place. DMAs and the first 16 abs/sign run under fc1_c0.
                a2row_ps = pr2.tile([1, D], F32, tag="a2row")

                def w2_dma_all():
                    for k in range(KH):
                        nc.sync.dma_start(w2b[:, k, :], w2t_3d[k])

                def w2_absign(k):
                    ab2 = a2pool.tile([128, D], BF16, tag="absw2",
                                      name=f"ab2_{k}")
                    nc.vector.tensor_scalar(
                        ab2[:].bitcast(mybir.dt.uint16),
                        w2b[:, k, :].bitcast(mybir.dt.uint16),
                        0x7FFF,
                        None,
                        ALU.bitwise_and,
                    )
                    # sign on DVE (ScalarE is loaded with gelus + w1 signs):
                    # bf16 sign(w) == (w & 0x8000) | 0x3f80, in place, ordered
                    # after the abs read by same-engine program order.
                    nc.vector.tensor_scalar(
                        w2b[:, k, :].bitcast(mybir.dt.uint16),
                        w2b[:, k, :].bitcast(mybir.dt.uint16),
                        0x8000,
                        0x3F80,
                        ALU.bitwise_and,
                        ALU.bitwise_or,
                    )
                    return ab2

                w2_dma_all()
                ab2_tiles = {}
                for k in range(16):
                    ab2_tiles[k] = w2_absign(k)

                for c in range(N_CHUNK):
                    csl = slice(c * TC, (c + 1) * TC)
                    with nc.named_scope(f"fc1_c{c}"):
                        xc = xpool.tile([128, KD, TC], BF16, tag="xc")
                        nc.sync.dma_start(xc[:], xt_3d[:, :, csl])
                        ht = hpool.tile([128, KH, TC], BF16, tag="ht")
                        for m in range(KH):
                            ps = ps1pool.tile([128, TC], F32, tag="ps1")
                            for k in range(KD):
                                nc.tensor.matmul(
                                    ps[:],
                                    lhsT=w1b[:, k, ts(m, 128)],
                                    rhs=xc[:, k, :],
                                    start=(k == 0),
                                    stop=(k == KD - 1),
                                )
                            nc.scalar.activation(
                                ht[:, m, :],
                                ps[:],
                                AF.Gelu,
                                bias=b1c[:, m : m + 1],
                                scale=alpha1c[:, m : m + 1],
                            )
                    if c == 0:
                        # alpha2: ones-stationary matmuls over |w2| slot in
                        # between fc1_c0 and fc2_c0 on the PE; the late abs
                        # tiles recycle the 16 scratch slots as they drain.
                        with nc.named_scope("w2prep"):
                            for k in range(KH):
                                ab2 = ab2_tiles.pop(k, None)
                                if ab2 is None:
                                    ab2 = w2_absign(k)
                                for n in range(D // 512):
                                    nc.tensor.matmul(
                                        a2row_ps[:, ts(n, 512)],
                                        lhsT=ones2[:],
                                        rhs=ab2[:, ts(n, 512)],
                                        start=(k == 0),
                                        stop=(k == KH - 1),
                                        skip_group_check=True,
                                    )
                            a2row = r2pool.tile([1, D], F32, tag="a2row_sb")
                            nc.vector.tensor_copy(out=a2row[:], in_=a2row_ps[:])
                            nc.sync.dma_start(a2d[:], a2row[:])
                            nc.sync.dma_start(
                                alpha2c[:], a2d.rearrange("(m p) -> p m", p=128)
                            )
                    with nc.named_scope(f"fc2_c{c}"):
                        for md in range(KD):
                            ps2 = ps2pool.tile([128, TC], F32, tag="ps2")
                            for mh in range(KH):
                                nc.tensor.matmul(
                                    ps2[:],
                                    lhsT=w2b[:, mh, ts(md, 128)],
                                    rhs=ht[:, mh, :],
                                    start=(mh == 0),
                                    stop=(mh == KH - 1),
                                )
                            oc = opool.tile([128, TC], F32, tag="oc")
                            nc.scalar.activation(
                                oc[:],
                                ps2[:],
                                AF.Identity,
                                bias=b2c[:, md : md + 1],
                                scale=alpha2c[:, md : md + 1],
                            )
                            nc.sync.dma_start(out_3d[md][:, csl], oc[:])

    nc.compile()
    return nc


_NC_CACHE = None


def _get_nc():
    global _NC_CACHE
    if _NC_CACHE is None:
        _NC_CACHE = build_kernel()
    return _NC_CACHE


def kernel(x, w1, b1, w2, b2):
    assert x.shape == (B, S, D) and w1.shape == (H, D) and w2.shape == (D, H)
    nc = _get_nc()

    import ml_dtypes

    xt = np.ascontiguousarray(x.reshape(T_GLOBAL, D).T).astype(
        ml_dtypes.bfloat16
    )                                                         # [D, 12544]
    w1t = np.ascontiguousarray(w1.T).astype(ml_dtypes.bfloat16)   # [D, H]
    w2t = np.ascontiguousarray(w2.T).astype(ml_dtypes.bfloat16)   # [H, D]
    b1 = np.ascontiguousarray(b1, dtype=np.float32)
    b2 = np.ascontiguousarray(b2, dtype=np.float32)

    in_maps = [
        {
            "xt": np.ascontiguousarray(xt[:, i * T : (i + 1) * T]),
            "w1t": w1t,
            "b1": b1,
            "w2t": w2t,
            "b2": b2,
        }
        for i in range(N_CORES)
    ]

    trace = bool(int(os.environ.get("BIMLP_TRACE", "0")))
    res = run_bass_kernel_spmd(
        nc, in_maps, core_ids=list(range(N_CORES)), trace=trace
    )
    if trace:
        kernel.last_results = res

    outt = np.concatenate([res.results[i]["out"] for i in range(N_CORES)], axis=1)
    return np.ascontiguousarray(outt.T).reshape(B, S, D).astype(np.float32)
